# revision 1
# baseline (speedup 1.0000x reference)
"""Dinov3 self-attention Bass kernel for TRN2.

Sharding: data-parallel over batch. B=8 batch elements -> 8 NeuronCores,
one full attention per core, weights replicated. No collectives.

Per-core layout strategy (all matmuls bf16 x bf16 -> fp32 PSUM):
  xT  [h, s]   : x cast to bf16, DMA-transposed           (h on partitions)
  WqT/WkT/WvT/WpT [h, o] : weights cast + DMA-transposed
  qT/kT [o, s] : projections computed transposed, RoPE'd
  v    [s, o]  : projection computed natural (M = s)
  scores.T [j, i] = kT^T @ qT per head (K=d=64, two heads row-packed)
  expS.T = exp(0.125 * scores.T) on ScalarE, psum -> sbuf bf16
  PV: ctx_u.T[d, i] = (v_h | ones)^T @ expS.T  -> row 64 = softmax denominator
  normalize: ctxT = ctx_u.T * bcast(1/denom)   (DVE + DMA partition-broadcast)
  out[i, o] = ctxT^T @ WpT (+ ones x bp)       -> fp32 -> DRAM
"""

import contextlib
import os
import sys

import numpy as np

sys.path.insert(0, "/opt/trn_rl_repo")

import concourse.bacc as bacc
import concourse.bass as bass
import concourse.tile as tile
from concourse import mybir

S = 1374
H = 768
NH = 12
D = 64
NROT = 1369
PREFIX = S - NROT  # 5
B = 8

P = 128
NSTILE = (S + P - 1) // P  # 11 s-tiles, last has 94 rows
NOTILE = H // P  # 6
SPAD = NSTILE * P  # 1408
ICHUNKS = ((0, 687), (687, 687))  # i-chunks, 2 psum banks each
NCHUNK = len(ICHUNKS)
BANK = 512  # fp32 elements per psum bank (matmul N limit)
SCR_W = 768  # padded width of the denominator scratch rows

F32 = mybir.dt.float32
BF16 = mybir.dt.bfloat16


def _subchunks(total):
    """Split a free-dim range into <=BANK pieces aligned to bank boundaries."""
    out = []
    off = 0
    while off < total:
        n = min(BANK, total - off)
        out.append((off, n))
        off += n
    return out


def _stile(i):
    """(start, size) of s-tile i."""
    start = i * P
    return start, min(P, S - start)


def build_kernel(nc):
    x_ext = nc.declare_dram_parameter("hidden_states", [S, H], F32, isOutput=False)
    sin_ext = nc.declare_dram_parameter("sin", [NROT, D], F32, isOutput=False)
    cos_ext = nc.declare_dram_parameter("cos", [NROT, D], F32, isOutput=False)
    wq_ext = nc.declare_dram_parameter("Wq", [H, H], F32, isOutput=False)
    bq_ext = nc.declare_dram_parameter("bq", [H], F32, isOutput=False)
    wk_ext = nc.declare_dram_parameter("Wk", [H, H], F32, isOutput=False)
    wv_ext = nc.declare_dram_parameter("Wv", [H, H], F32, isOutput=False)
    bv_ext = nc.declare_dram_parameter("bv", [H], F32, isOutput=False)
    wp_ext = nc.declare_dram_parameter("Wp", [H, H], F32, isOutput=False)
    bp_ext = nc.declare_dram_parameter("bp", [H], F32, isOutput=False)
    out_ext = nc.declare_dram_parameter("out", [S, H], F32, isOutput=True)

    with tile.TileContext(nc) as tc:
        _body(tc, x_ext, sin_ext, cos_ext, wq_ext, bq_ext, wk_ext,
              wv_ext, bv_ext, wp_ext, bp_ext, out_ext)
    nc.compile()
    return nc


def _body(tc, x_ext, sin_ext, cos_ext, wq_ext, bq_ext, wk_ext, wv_ext,
          bv_ext, wp_ext, bp_ext, out_ext):
    nc = tc.nc
    from concourse.masks import make_identity

    with contextlib.ExitStack() as ctx:
        # ---------------- long-lived pools ----------------
        persist = ctx.enter_context(tc.tile_pool(name="persist", bufs=1))
        psum_qk = ctx.enter_context(tc.tile_pool(name="psum_qk", bufs=2, space="PSUM"))
        psum_pv = ctx.enter_context(tc.tile_pool(name="psum_pv", bufs=2, space="PSUM"))

        xT = persist.tile([P, NOTILE, SPAD], BF16)     # xT[p, t, s] = x[s, 128t+p]
        wpT = persist.tile([P, NOTILE, H], BF16)
        qT = persist.tile([P, NOTILE, SPAD], BF16)     # roped q, [o, s] layout
        kT = persist.tile([P, NOTILE, SPAD], BF16)
        # v[s, (h, d|1)]: per head 64 v columns + a ones column, so the PV
        # matmul computes ctx rows AND the softmax denominator in one M=65 MM
        vsb = persist.tile([P, NSTILE, NH, D + 1], BF16)
        ctxT = persist.tile([P, NOTILE, SPAD], BF16)   # ctx^T [(h,d), i]
        cc2 = persist.tile([P, SPAD], BF16)            # cos^T stacked twice
        ss2 = persist.tile([P, SPAD], BF16)            # sin^T stacked, sign-baked
        bq_sb = persist.tile([P, NOTILE], F32)
        bv_row = persist.tile([1, H], BF16)
        bp_row = persist.tile([1, H], BF16)
        ones_row = persist.tile([1, P], BF16)          # K=1 bias matmuls (lhsT)

        nc.vector.memset(ones_row, 1.0)
        # ones columns of vsb (index 64 of each head's slot)
        nc.vector.memset(vsb[:, :, :, D:D + 1], 1.0)

        with tc.tile_pool(name="wqkv", bufs=1) as wqkv_pool, \
             tc.tile_pool(name="ropet", bufs=3) as ropet, \
             tc.tile_pool(name="setup_stage", bufs=2) as stage:
            wqT = wqkv_pool.tile([P, NOTILE, H], BF16)
            wkT = wqkv_pool.tile([P, NOTILE, H], BF16)
            wvT = wqkv_pool.tile([P, NOTILE, H], BF16)

            # ---------------- biases ----------------
            # bq as [128, 6]: column t = bq[128t : 128t+128]
            nc.sync.dma_start(out=bq_sb,
                              in_=bq_ext.rearrange("(t p) -> p t", p=P))
            bstage = stage.tile([1, H], F32, tag="bias_stage", bufs=1)
            nc.sync.dma_start(out=bstage,
                              in_=bv_ext.rearrange("(a h) -> a h", a=1))
            nc.vector.tensor_copy(out=bv_row, in_=bstage)
            bstage2 = stage.tile([1, H], F32, tag="bias_stage2", bufs=1)
            nc.sync.dma_start(out=bstage2,
                              in_=bp_ext.rearrange("(a h) -> a h", a=1))
            nc.vector.tensor_copy(out=bp_row, in_=bstage2)

            def load_weight(w_ext, wT):
                for r in range(NOTILE):  # row tile of W (o dim)
                    ws = stage.tile([P, H], F32, tag="w_stage", name=f"ws_{r}")
                    wb = stage.tile([P, H], BF16, tag="w_stage_bf", name=f"wb_{r}")
                    nc.sync.dma_start(out=ws, in_=w_ext[r * P:(r + 1) * P, :])
                    nc.vector.tensor_copy(out=wb, in_=ws)
                    nc.scalar.dma_start_transpose(
                        out=wT[:, :, r * P:(r + 1) * P], in_=wb)

            def qk_proj(wT, dstT, bias):
                for ot in range(NOTILE):
                    qb = ropet.tile([P, SPAD], BF16, tag="qb", name=f"qb_{ot}")
                    for (i0, ilen) in ICHUNKS:
                        ps = psum_qk.tile([P, 2 * BANK], F32, tag="qk",
                                          name="qkps")[:, :ilen]
                        for kt in range(NOTILE):
                            for (o, n) in _subchunks(ilen):
                                nc.tensor.matmul(
                                    ps[:, o:o + n],
                                    wT[:, kt, ot * P:(ot + 1) * P],
                                    xT[:, kt, i0 + o:i0 + o + n],
                                    start=(kt == 0), stop=(kt == NOTILE - 1))
                        # evict + bias (per-partition scalar) -> bf16 on ACT
                        # (idle during the projection phase)
                        if bias:
                            nc.scalar.add(qb[:, i0:i0 + ilen], ps,
                                          bq_sb[:, ot:ot + 1])
                        else:
                            nc.scalar.copy(out=qb[:, i0:i0 + ilen], in_=ps)
                    # RoPE: rot[p] = qb[pair(p)] via partition-shifted DMA
                    rot = ropet.tile([P, NROT], BF16, tag="rot", name=f"rot_{ot}")
                    for (dst0, src0) in ((0, 32), (32, 0), (64, 96), (96, 64)):
                        nc.sync.dma_start(
                            out=rot[dst0:dst0 + 32, :],
                            in_=qb[src0:src0 + 32, PREFIX:PREFIX + NROT])
                    sl = slice(PREFIX, PREFIX + NROT)
                    nc.vector.tensor_mul(dstT[:, ot, sl], qb[:, sl],
                                         cc2[:, :NROT])
                    nc.vector.tensor_mul(rot, rot, ss2[:, :NROT])
                    nc.vector.tensor_add(dstT[:, ot, sl], dstT[:, ot, sl], rot)
                    nc.vector.tensor_copy(out=dstT[:, ot, 0:PREFIX],
                                          in_=qb[:, 0:PREFIX])

            # emission order = DMA queue order: Wq, x, (q-proj), Wk, (k-proj),
            # Wv, (v-proj), Wp -- gets the PE going as early as possible
            load_weight(wq_ext, wqT)

            # ---------------- load & transpose x ----------------
            for st in range(NSTILE):
                s0, ssz = _stile(st)
                xs = stage.tile([P, H], F32, tag="x_stage", name=f"xs_{st}")
                xb = stage.tile([P, H], BF16, tag="x_stage_bf", name=f"xb_{st}")
                if ssz < P:
                    nc.vector.memset(xb, 0.0)
                nc.sync.dma_start(out=xs[:ssz], in_=x_ext[s0:s0 + ssz, :])
                nc.vector.tensor_copy(out=xb[:ssz], in_=xs[:ssz])
                nc.scalar.dma_start_transpose(out=xT[:, :, s0:s0 + P], in_=xb)

            # ---------------- sin/cos tables ----------------
            # [NROT, 64] f32 -> bf16 [64, s]: stack 11 row-tiles side by side
            # (padded to 128 cols each) and transpose them all with ONE
            # batched xbar DMA, then copy into both halves of [128, s].
            n_rtile = (NROT + P - 1) // P
            for src_ext, dstT in ((cos_ext, cc2), (sin_ext, ss2)):
                cst_all = stage.tile([P, SPAD], F32, tag="cs_stage")
                csb_all = stage.tile([P, SPAD], BF16, tag="cs_stage_bf")
                csT3 = stage.tile([P, n_rtile, P], BF16, tag="cs_T3")
                nc.vector.memset(csb_all, 0.0)
                for i in range(n_rtile):
                    r0 = i * P
                    rsz = min(P, NROT - r0)
                    nc.sync.dma_start(out=cst_all[:rsz, i * P:i * P + D],
                                      in_=src_ext[r0:r0 + rsz, :])
                    nc.vector.tensor_copy(
                        out=csb_all[:rsz, i * P:i * P + D],
                        in_=cst_all[:rsz, i * P:i * P + D])
                nc.scalar.dma_start_transpose(out=csT3, in_=csb_all)
                for i in range(n_rtile):
                    r0 = i * P
                    rsz = min(P, NROT - r0)
                    for half in range(2):
                        nc.scalar.copy(
                            out=dstT[64 * half:64 * half + 64, r0:r0 + rsz],
                            in_=csT3[0:D, i, :rsz])
            # bake rotate_half sign into ss2: rows 0:32 and 64:96 negated
            for base in (0, 64):
                sl = slice(base, base + 32)
                nc.vector.tensor_scalar_mul(ss2[sl, :NROT],
                                            ss2[sl, :NROT], -1.0)


            qk_proj(wqT, qT, True)
            load_weight(wk_ext, wkT)
            qk_proj(wkT, kT, False)
            load_weight(wv_ext, wvT)

            # ---------------- v projection (natural out) ----------------
            for st in range(NSTILE):
                s0, ssz = _stile(st)
                for ci, (o, n) in enumerate(_subchunks(H)):
                    ps = psum_pv.tile([P, 2 * BANK], F32, tag="pv",
                                      name=f"vps_{st}_{ci}")[:, :n]
                    for kt in range(NOTILE):
                        nc.tensor.matmul(
                            ps[:ssz, :],
                            xT[:, kt, s0:s0 + ssz],
                            wvT[:, kt, o:o + n],
                            start=(kt == 0), stop=False)
                    # bias: += ones[s] x bv[o]  (K=1 rank-1 update ends group)
                    nc.tensor.matmul(
                        ps[:ssz, :],
                        ones_row[:, :ssz],
                        bv_row[:, o:o + n],
                        start=False, stop=True)
                    # scatter heads into their 65-wide slots (8 heads per 512)
                    nc.scalar.copy(
                        out=vsb[:ssz, st, o // D:(o + n) // D, 0:D],
                        in_=ps[:ssz, :].rearrange("p (h d) -> p h d", d=D))

            load_weight(wp_ext, wpT)

        # ---------------- attention (6 head pairs) ----------------
        exps_pool = ctx.enter_context(tc.tile_pool(name="exps_pool", bufs=6))
        norm_pool = ctx.enter_context(tc.tile_pool(name="norm_pool", bufs=6))
        outst = ctx.enter_context(tc.tile_pool(name="outst", bufs=2))
        dram_pool = ctx.enter_context(
            tc.tile_pool(name="dram_pool", bufs=1, space="DRAM"))
        rs_scratch = dram_pool.tile([NH * NCHUNK, SCR_W], F32)  # 36 rows
        # prefill with 1.0 so the 687:768 pad cols stay finite under recip
        ones_f32 = norm_pool.tile([1, SCR_W], F32, tag="ones_f32", bufs=1)
        nc.vector.memset(ones_f32, 1.0)
        for idx in range(NH * NCHUNK):
            nc.sync.dma_start(out=rs_scratch[idx:idx + 1, :], in_=ones_f32)

        for pt in range(NOTILE):  # head pair = heads (2pt, 2pt+1)
            for c, (i0, ilen) in enumerate(ICHUNKS):
                pv_ps = []
                for hh in range(2):
                    pv_ps.append(psum_pv.tile([P, 2 * BANK], F32, tag="pv",
                                              name=f"pvps_{pt}_{c}_{hh}")[:, :ilen])
                for jt in range(NSTILE):
                    j0, jsz = _stile(jt)
                    exps = []
                    for hh in range(2):  # head half: partitions 64*hh
                        hb = 64 * hh
                        sc = psum_qk.tile([P, 2 * BANK], F32, tag="qk",
                                          name=f"scps_{pt}_{c}_{jt}_{hh}")[:, :ilen]
                        for (o, n) in _subchunks(ilen):
                            nc.tensor.matmul(
                                sc[:jsz, o:o + n],
                                kT[hb:hb + 64, pt, j0:j0 + jsz],
                                qT[hb:hb + 64, pt, i0 + o:i0 + o + n],
                                start=True, stop=True)
                        es = exps_pool.tile([P, 2 * BANK], BF16, tag="es",
                                            name=f"es_{pt}_{c}_{jt}_{hh}")
                        nc.scalar.activation(
                            out=es[:jsz, :ilen], in_=sc[:jsz, :],
                            func=mybir.ActivationFunctionType.Exp,
                            scale=float(D) ** -0.5)
                        exps.append(es)
                    for hh in range(2):
                        h = 2 * pt + hh
                        es = exps[hh]
                        # ctx_u^T rows 0:64 + denominator row 64, one MM
                        for (o, n) in _subchunks(ilen):
                            nc.tensor.matmul(
                                pv_ps[hh][0:D + 1, o:o + n],
                                vsb[:jsz, jt, h, :],
                                es[:jsz, o:o + n],
                                start=(jt == 0), stop=(jt == NSTILE - 1))
                # normalize: ctxT = ctx_u^T * (1/denom), bcast over partitions.
                # 1. evict psum to sbuf immediately (frees the pv psum slot).
                # 2. reciprocal is free-dim-serial (~8 cyc/elem), so reshape
                #    the denom row into [128, 6] via a DRAM bounce and run
                #    the recip across partitions instead.
                # 3. SBUF APs can't have partition step 0, DRAM APs can --
                #    broadcast-read the recip'd row from the DRAM scratch.
                for hh in range(2):
                    idx = (pt * 2 + hh) * NCHUNK + c
                    stg = norm_pool.tile([D + 1, 2 * BANK], F32, tag="stg",
                                         name=f"stg_{pt}_{c}_{hh}")[:, :ilen]
                    nc.vector.tensor_copy(out=stg, in_=pv_ps[hh][0:D + 1, :])
                    nc.sync.dma_start(out=rs_scratch[idx:idx + 1, :ilen],
                                      in_=stg[D:D + 1, :])
                    rsh = norm_pool.tile([P, SCR_W // P], F32, tag="rsh",
                                         name=f"rsh_{pt}_{c}_{hh}")
                    nc.sync.dma_start(
                        out=rsh, in_=rs_scratch[idx, :].rearrange(
                            "(i p) -> p i", p=P))
                    nc.vector.reciprocal(out=rsh, in_=rsh)
                    nc.sync.dma_start(
                        out=rs_scratch[idx, :].rearrange("(i p) -> p i", p=P),
                        in_=rsh)
                    bc = norm_pool.tile([D, 2 * BANK], F32, tag="bc",
                                        name=f"bc_{pt}_{c}_{hh}")[:, :ilen]
                    scr_row = rs_scratch[idx:idx + 1, :ilen]
                    bcast_src = bass.AP(
                        tensor=scr_row.tensor, offset=scr_row.offset,
                        ap=[[0, D]] + list(scr_row.ap[1:]))
                    nc.sync.dma_start(out=bc, in_=bcast_src)
                    nc.vector.tensor_mul(
                        ctxT[64 * hh:64 * hh + 64, pt, i0:i0 + ilen],
                        stg[0:D, :], bc)

        # ---------------- output projection ----------------
        for it in range(NSTILE):
            s0, ssz = _stile(it)
            ot = outst.tile([P, H], F32, tag="ostage", name=f"ost_{it}")
            for ci, (o, n) in enumerate(_subchunks(H)):
                ps = psum_pv.tile([P, 2 * BANK], F32, tag="pv",
                                  name=f"ops_{it}_{ci}")[:, :n]
                for kt in range(NOTILE):
                    nc.tensor.matmul(
                        ps[:ssz, :],
                        ctxT[:, kt, s0:s0 + ssz],
                        wpT[:, kt, o:o + n],
                        start=(kt == 0), stop=False)
                nc.tensor.matmul(
                    ps[:ssz, :],
                    ones_row[:, :ssz],
                    bp_row[:, o:o + n],
                    start=False, stop=True)
                nc.scalar.copy(out=ot[:ssz, o:o + n], in_=ps[:ssz])
            nc.sync.dma_start(out=out_ext[s0:s0 + ssz, :], in_=ot[:ssz])


_NC_CACHE = None


def get_nc():
    global _NC_CACHE
    if _NC_CACHE is None:
        nc = bacc.Bacc(None, target_bir_lowering=False, debug=False)
        _NC_CACHE = build_kernel(nc)
    return _NC_CACHE


def kernel(**inputs):
    from concourse.bass_utils import run_bass_kernel_spmd

    nc = get_nc()
    names = ["hidden_states", "sin", "cos", "Wq", "bq", "Wk", "Wv", "bv", "Wp", "bp"]
    arrs = {k: np.ascontiguousarray(np.asarray(inputs[k], dtype=np.float32))
            for k in names}
    in_maps = []
    for b in range(B):
        m = {k: arrs[k] for k in names if k != "hidden_states"}
        m["hidden_states"] = np.ascontiguousarray(arrs["hidden_states"][b])
        in_maps.append(m)
    res = run_bass_kernel_spmd(nc, in_maps, core_ids=list(range(B)))
    out = np.stack([res.results[b]["out"] for b in range(B)], axis=0)
    return out.astype(np.float32)


if __name__ == "__main__":
    # quick smoke: build only
    nc = get_nc()
    print("built ok")



# revision 4
# speedup vs baseline: 1.9891x; 1.9891x over previous
"""Dinov3 self-attention Bass kernel for TRN2 (v3).

Sharding: data-parallel over batch. B=8 batch elements -> 8 NeuronCores,
one full attention per core, weights replicated. No collectives.

v3 vs v2 (512us):
  - Host-side bf16 pre-cast of x/W/sin/cos/bv/bp: halves DRAM load
    bytes and removes EVERY on-device cast (v2 burned 62us of GpSimd
    and ~25us of DVE on casts, serializing the weight pipeline).
  - Direct bf16 loads -> single batched xbar transpose per weight;
    x transposed in 3 batched xbar DMAs into a 4D xT layout.
  - Weight transposes issued on the sync ring, x/cs transposes on the
    ACT ring (two HW-DGE rings run concurrently).
  - Weight loads interleaved with projection emission so the rot
    (RoPE partition-shift) DMAs aren't FIFO-blocked behind Wv/Wp.
Attention (unchanged from v2): i-chunks {512,512,350}; pair score
tile [128,2,512] double-buffered (4 banks) + PV pair accumulators
with ones-column denominator (4 banks); one exp ACT instruction per
(pt,c,jt) spanning both heads; scores row-packed via tile_position;
normalize = reciprocal_approx_fast + gpsimd partition_broadcast.
"""

import contextlib
import sys

import numpy as np

sys.path.insert(0, "/opt/trn_rl_repo")

import concourse.bacc as bacc
import concourse.bass as bass
import concourse.tile as tile
from concourse import mybir

S = 1374
H = 768
NH = 12
D = 64
NROT = 1369
PREFIX = S - NROT  # 5
B = 8

P = 128
NSTILE = (S + P - 1) // P  # 11 s-tiles, last has 94 rows
NOTILE = H // P  # 6
SPAD = NSTILE * P  # 1408
IC = ((0, 512), (512, 512), (1024, 350))  # i-chunks (psum-bank sized)
OC = ((0, 512), (512, 256))  # o-chunks for v/out projections

F32 = mybir.dt.float32
BF16 = mybir.dt.bfloat16
EXP = mybir.ActivationFunctionType.Exp


def _stile(i):
    start = i * P
    return start, min(P, S - start)


def build_kernel(nc):
    x_ext = nc.declare_dram_parameter("hidden_states", [S, H], BF16, isOutput=False)
    sin_ext = nc.declare_dram_parameter("sin", [NROT, D], BF16, isOutput=False)
    cos_ext = nc.declare_dram_parameter("cos", [NROT, D], BF16, isOutput=False)
    wq_ext = nc.declare_dram_parameter("Wq", [H, H], BF16, isOutput=False)
    bq_ext = nc.declare_dram_parameter("bq", [H], F32, isOutput=False)
    wk_ext = nc.declare_dram_parameter("Wk", [H, H], BF16, isOutput=False)
    wv_ext = nc.declare_dram_parameter("Wv", [H, H], BF16, isOutput=False)
    bv_ext = nc.declare_dram_parameter("bv", [H], BF16, isOutput=False)
    wp_ext = nc.declare_dram_parameter("Wp", [H, H], BF16, isOutput=False)
    bp_ext = nc.declare_dram_parameter("bp", [H], BF16, isOutput=False)
    out_ext = nc.declare_dram_parameter("out", [S, H], F32, isOutput=True)

    with tile.TileContext(nc) as tc:
        _body(tc, x_ext, sin_ext, cos_ext, wq_ext, bq_ext, wk_ext,
              wv_ext, bv_ext, wp_ext, bp_ext, out_ext)
    nc.compile()
    return nc


def _body(tc, x_ext, sin_ext, cos_ext, wq_ext, bq_ext, wk_ext, wv_ext,
          bv_ext, wp_ext, bp_ext, out_ext):
    nc = tc.nc

    with contextlib.ExitStack() as ctx:
        persist = ctx.enter_context(tc.tile_pool(name="persist", bufs=1))

        xT = persist.tile([P, NSTILE, NOTILE, P], BF16)  # xT[hc, st, kt, sc]
        qT = persist.tile([P, NOTILE, SPAD], BF16)       # roped q, [o, s]
        kT = persist.tile([P, NOTILE, SPAD], BF16)
        ctxT = persist.tile([P, NOTILE, SPAD], BF16)     # ctx^T [(h,d), i]
        # v[s, (h, d|1)]: per head 64 v cols + ones col -> PV computes
        # ctx rows AND the softmax denominator in one M=65 matmul
        vsb = persist.tile([P, NSTILE, NH, D + 1], BF16)
        wqT = persist.tile([P, NOTILE, NOTILE, P], BF16)  # [hc, ot, kt, oc]
        wkT = persist.tile([P, NOTILE, NOTILE, P], BF16)
        wvT = persist.tile([P, NOTILE, NOTILE, P], BF16)
        wpT = persist.tile([P, NOTILE, NOTILE, P], BF16)
        cc2 = persist.tile([P, SPAD], BF16)              # cos^T stacked twice
        ss2 = persist.tile([P, SPAD], BF16)              # sin^T, sign-baked
        bq_sb = persist.tile([P, NOTILE], F32)
        bv_row = persist.tile([1, H], BF16)
        bp_row = persist.tile([1, H], BF16)
        ones_row = persist.tile([1, P], BF16)

        nc.vector.memset(ones_row, 1.0)
        nc.vector.memset(vsb[:, :, :, D:D + 1], 1.0)

        with tc.tile_pool(name="stage", bufs=1) as stage, \
             tc.tile_pool(name="ropet", bufs=2) as ropet, \
             tc.tile_pool(name="proj_psum", bufs=2, space="PSUM") as pps:
            # exp table preload on ACT while DMAs run
            dmy = stage.tile([1, 16], F32, tag="dmy")
            dmo = stage.tile([1, 16], BF16, tag="dmo")
            nc.vector.memset(dmy, 0.0)
            nc.scalar.activation(out=dmo, in_=dmy, func=EXP, scale=0.125)

            # biases (bf16 on host already; bq stays f32 for per-part add)
            nc.sync.dma_start(out=bq_sb,
                              in_=bq_ext.rearrange("(t p) -> p t", p=P))
            nc.sync.dma_start(out=bv_row,
                              in_=bv_ext.rearrange("(a h) -> a h", a=1))
            nc.sync.dma_start(out=bp_row,
                              in_=bp_ext.rearrange("(a h) -> a h", a=1))

            wname = [0]

            def load_weight(w_ext, wT, teng):
                # bf16 load + one batched xbar transpose
                wname[0] += 1
                wb = stage.tile([P, NOTILE, H], BF16, tag="wb", bufs=2,
                                name=f"wb_{wname[0]}")
                nc.sync.dma_start(
                    out=wb, in_=w_ext.rearrange("(t p) h -> p t h", p=P))
                # out[hc, t, hb, p] = W[t*128+p, hb*128+hc]
                teng.dma_start_transpose(out=wT, in_=wb)

            load_weight(wq_ext, wqT, nc.scalar)

            # x: bf16 batched loads + batched xbar transposes (ACT ring)
            for b0 in range(0, NSTILE, 4):
                nb = min(4, NSTILE - b0)
                nfull = min(nb, S // P - b0)
                xb = stage.tile([P, 4, H], BF16, tag="xb", bufs=2,
                                name=f"xb_{b0}")
                if nfull:
                    nc.sync.dma_start(
                        out=xb[:, :nfull, :],
                        in_=x_ext[b0 * P:(b0 + nfull) * P, :].rearrange(
                            "(t p) h -> p t h", p=P))
                if nfull < nb:
                    s0 = (b0 + nfull) * P
                    nc.sync.dma_start(out=xb[:S - s0, nfull, :],
                                      in_=x_ext[s0:S, :])
                nc.scalar.dma_start_transpose(
                    out=xT[:, b0:b0 + nb, :, :], in_=xb[:, :nb, :])

            # sin/cos -> cc2/ss2 [128, s] bf16 (two stacked 64-row halves)
            n_rtile = (NROT + P - 1) // P  # 11
            for ti, (src_ext, dstT) in enumerate(((cos_ext, cc2), (sin_ext, ss2))):
                csb = stage.tile([P, n_rtile, P], BF16, tag="cs_stage",
                                 bufs=2, name=f"csb_{ti}")
                csT3 = stage.tile([P, n_rtile, P], BF16, tag="cs_T3",
                                  bufs=2, name=f"csT_{ti}")
                nfull = NROT // P  # 10
                nc.sync.dma_start(
                    out=csb[:, :nfull, :D],
                    in_=src_ext[:nfull * P, :].rearrange(
                        "(t p) d -> p t d", p=P))
                nc.sync.dma_start(out=csb[:NROT - nfull * P, nfull, :D],
                                  in_=src_ext[nfull * P:, :])
                nc.scalar.dma_start_transpose(out=csT3, in_=csb)
                for i in range(n_rtile):
                    r0 = i * P
                    rsz = min(P, NROT - r0)
                    for half in range(2):
                        nc.vector.tensor_copy(
                            out=dstT[64 * half:64 * half + 64, r0:r0 + rsz],
                            in_=csT3[0:D, i, :rsz])
            # bake rotate_half sign into ss2: rows 0:32 and 64:96 negated
            for base in (0, 64):
                sl = slice(base, base + 32)
                nc.vector.tensor_scalar_mul(ss2[sl, :NROT],
                                            ss2[sl, :NROT], -1.0)

            load_weight(wk_ext, wkT, nc.sync)

            # ---------------- projections ----------------
            def qk_proj(wT, dstT, bias, tag):
                for ot in range(NOTILE):
                    qb = ropet.tile([P, SPAD], BF16, tag="qb",
                                    name=f"qb_{tag}_{ot}")
                    for ci, (i0, ilen) in enumerate(IC):
                        ps = pps.tile([P, 512], F32, tag="p",
                                      name=f"qkps_{tag}_{ot}_{ci}")[:, :ilen]
                        st0 = i0 // P
                        for kt in range(NOTILE):
                            if ilen == 512:
                                nc.tensor.matmul(
                                    ps, wT[:, ot, kt, :],
                                    xT[:, st0:st0 + 4, kt, :],
                                    start=(kt == 0), stop=(kt == NOTILE - 1))
                            else:  # 350 = 256 + 94
                                nc.tensor.matmul(
                                    ps[:, 0:256], wT[:, ot, kt, :],
                                    xT[:, st0:st0 + 2, kt, :],
                                    start=(kt == 0), stop=(kt == NOTILE - 1))
                                nc.tensor.matmul(
                                    ps[:, 256:350], wT[:, ot, kt, :],
                                    xT[:, st0 + 2, kt, :94],
                                    start=(kt == 0), stop=(kt == NOTILE - 1))
                        if bias:
                            nc.vector.tensor_scalar_add(
                                qb[:, i0:i0 + ilen], ps, bq_sb[:, ot:ot + 1])
                        else:
                            nc.vector.tensor_copy(
                                out=qb[:, i0:i0 + ilen], in_=ps)
                    # RoPE: rot[p] = qb[pair(p)] via partition-shift DMA
                    rot = ropet.tile([P, SPAD], BF16, tag="rot",
                                     name=f"rot_{tag}_{ot}")
                    for (dst0, src0) in ((0, 32), (32, 0), (64, 96), (96, 64)):
                        nc.sync.dma_start(
                            out=rot[dst0:dst0 + 32, :NROT],
                            in_=qb[src0:src0 + 32, PREFIX:PREFIX + NROT])
                    sl = slice(PREFIX, PREFIX + NROT)
                    nc.vector.tensor_mul(dstT[:, ot, sl], qb[:, sl],
                                         cc2[:, :NROT])
                    nc.vector.tensor_mul(rot[:, :NROT], rot[:, :NROT],
                                         ss2[:, :NROT])
                    nc.vector.tensor_add(dstT[:, ot, sl], dstT[:, ot, sl],
                                         rot[:, :NROT])
                    nc.vector.tensor_copy(out=dstT[:, ot, 0:PREFIX],
                                          in_=qb[:, 0:PREFIX])

            qk_proj(wqT, qT, True, "q")
            load_weight(wv_ext, wvT, nc.sync)
            qk_proj(wkT, kT, False, "k")
            load_weight(wp_ext, wpT, nc.sync)

            # v projection (natural out) + scatter into 65-wide slots
            for st in range(NSTILE):
                s0, ssz = _stile(st)
                for (o0, on) in OC:
                    ps = pps.tile([P, 512], F32, tag="p",
                                  name=f"vps_{st}_{o0}")[:ssz, :on]
                    for kt in range(NOTILE):
                        nc.tensor.matmul(
                            ps, xT[:, st, kt, :ssz],
                            wvT[:, o0 // P:(o0 + on) // P, kt, :],
                            start=(kt == 0), stop=False)
                    nc.tensor.matmul(
                        ps, ones_row[:, :ssz], bv_row[:, o0:o0 + on],
                        start=False, stop=True)
                    nc.vector.tensor_copy(
                        out=vsb[:ssz, st, o0 // D:(o0 + on) // D, 0:D],
                        in_=ps.rearrange("p (h d) -> p h d", d=D))

        # ---------------- attention ----------------
        scale = float(D) ** -0.5
        with tc.tile_pool(name="sc_psum", bufs=2, space="PSUM") as scp, \
             tc.tile_pool(name="pv_psum", bufs=2, space="PSUM") as pvp, \
             tc.tile_pool(name="es_pool", bufs=3) as esp, \
             tc.tile_pool(name="norm_pool", bufs=2) as nrm:
            for pt in range(NOTILE):
                for ci, (i0, ilen) in enumerate(IC):
                    pvt = pvp.tile([P, 2, 512], F32, tag="pv",
                                   name=f"pv_{pt}_{ci}")
                    sct = [None] * NSTILE

                    def emit_scores(jt):
                        j0, jsz = _stile(jt)
                        t = scp.tile([P, 2, 512], F32, tag="sc",
                                     name=f"sc_{pt}_{ci}_{jt}")
                        for hh in range(2):
                            hb = 64 * hh
                            nc.tensor.matmul(
                                t[:jsz, hh, :ilen],
                                kT[hb:hb + 64, pt, j0:j0 + jsz],
                                qT[hb:hb + 64, pt, i0:i0 + ilen],
                                start=True, stop=True)
                        sct[jt] = t

                    emit_scores(0)
                    emit_scores(1)
                    for jt in range(NSTILE):
                        j0, jsz = _stile(jt)
                        es = esp.tile([P, 2, 512], BF16, tag="es",
                                      name=f"es_{pt}_{ci}_{jt}")
                        # one ACT instr for both heads (3D psum AP)
                        nc.scalar.activation(
                            out=es[:jsz, :, :ilen],
                            in_=sct[jt][:jsz, :, :ilen],
                            func=EXP, scale=scale)
                        for hh in range(2):
                            nc.tensor.matmul(
                                pvt[0:D + 1, hh, :ilen],
                                vsb[:jsz, jt, 2 * pt + hh, :],
                                es[:jsz, hh, :ilen],
                                start=(jt == 0), stop=(jt == NSTILE - 1))
                        if jt + 2 < NSTILE:
                            emit_scores(jt + 2)

                    # normalize: ctxT = ctx_u * bcast(1/denom)
                    for hh in range(2):
                        dn = nrm.tile([1, 512], F32, tag="dn",
                                      name=f"dn_{pt}_{ci}_{hh}")[:, :ilen]
                        rc = nrm.tile([1, 512], F32, tag="rc",
                                      name=f"rc_{pt}_{ci}_{hh}")[:, :ilen]
                        bc = nrm.tile([D, 512], F32, tag="bc",
                                      name=f"bc_{pt}_{ci}_{hh}")[:, :ilen]
                        nc.vector.tensor_copy(out=dn, in_=pvt[D:D + 1, hh, :ilen])
                        nc.vector.reciprocal_approx_fast(out=rc, in_=dn)
                        nc.gpsimd.partition_broadcast(bc, rc)
                        nc.vector.tensor_mul(
                            ctxT[64 * hh:64 * hh + 64, pt, i0:i0 + ilen],
                            pvt[0:D, hh, :ilen], bc)

        # ---------------- output projection ----------------
        with tc.tile_pool(name="out_psum", bufs=2, space="PSUM") as ops, \
             tc.tile_pool(name="outst", bufs=2) as outst:
            for it in range(NSTILE):
                s0, ssz = _stile(it)
                ot = outst.tile([P, H], F32, tag="ostage", name=f"ost_{it}")
                for (o0, on) in OC:
                    ps = ops.tile([P, 512], F32, tag="o",
                                  name=f"ops_{it}_{o0}")[:ssz, :on]
                    for kt in range(NOTILE):
                        nc.tensor.matmul(
                            ps, ctxT[:, kt, s0:s0 + ssz],
                            wpT[:, o0 // P:(o0 + on) // P, kt, :],
                            start=(kt == 0), stop=False)
                    nc.tensor.matmul(
                        ps, ones_row[:, :ssz], bp_row[:, o0:o0 + on],
                        start=False, stop=True)
                    nc.vector.tensor_copy(out=ot[:ssz, o0:o0 + on], in_=ps)
                nc.sync.dma_start(out=out_ext[s0:s0 + ssz, :], in_=ot[:ssz])


_NC_CACHE = None


def get_nc():
    global _NC_CACHE
    if _NC_CACHE is None:
        nc = bacc.Bacc(None, target_bir_lowering=False, debug=False)
        _NC_CACHE = build_kernel(nc)
    return _NC_CACHE


def kernel(**inputs):
    import ml_dtypes
    from concourse.bass_utils import run_bass_kernel_spmd

    nc = get_nc()
    names = ["hidden_states", "sin", "cos", "Wq", "bq", "Wk", "Wv", "bv", "Wp", "bp"]
    bf16 = {"hidden_states", "sin", "cos", "Wq", "Wk", "Wv", "bv", "Wp", "bp"}
    arrs = {}
    for k in names:
        a = np.asarray(inputs[k], dtype=np.float32)
        if k in bf16:
            a = a.astype(ml_dtypes.bfloat16)
        arrs[k] = np.ascontiguousarray(a)
    in_maps = []
    for b in range(B):
        m = {k: arrs[k] for k in names if k != "hidden_states"}
        m["hidden_states"] = np.ascontiguousarray(arrs["hidden_states"][b])
        in_maps.append(m)
    res = run_bass_kernel_spmd(nc, in_maps, core_ids=list(range(B)))
    out = np.stack([res.results[b]["out"] for b in range(B)], axis=0)
    return out.astype(np.float32)


if __name__ == "__main__":
    nc = get_nc()
    print("built ok")
